# revision 29
# baseline (speedup 1.0000x reference)
"""Self-contained MiniSpinNet kernel for 8 Trainium2 NeuronCores.

kernel(**inputs) takes the FULL unsharded inputs (as produced by
setup_inputs()) and returns the full (2, 64, 256) float32 descriptor.

Data-parallel over the 128 B*M centers (16 per core).  The environment
charges a large flat cost per *instruction* (~80us/matmul, ~45us/vector
op), so the kernel is structured to minimize instruction count:
  - weights are shipped sharded 1/8-per-core and AllGathered on device
  - binning matmuls run in f32 (no hi/lo split, single instruction each)
  - conv1/conv2 pack 3/2 kernel taps into the 128-row contraction via
    column-shifted copies of the activations (sigma-shift packing)
  - mask building and drains are merged into wide single instructions
"""
import numpy as np
import ml_dtypes

import concourse.bass as bass
import concourse.bacc as bacc
import concourse.mybir as mybir
import concourse.tile as tile
from concourse import bass_utils

F32 = mybir.dt.float32
BF16 = mybir.dt.bfloat16
AF = mybir.ActivationFunctionType
OP = mybir.AluOpType
AX = mybir.AxisListType

B, N, M = 2, 2048, 64
BM = B * M
S = 16
NCORES = 8
NCH = 16
SK = S * NCH
EPS = 1e-5

COS_B = [float(np.float32(np.cos(j * np.pi / 8))) for j in range(1, 8)]
R2_B = [float(np.float32((j / 16.0) ** 2)) for j in range(1, 8)]

PD = 1024
PADW = S * PD

# blob layout (bf16 elements): aux f32 pairs, then compact bf16 weights
AUX_F32 = 1712                     # wr4 384 | gb 1280 | thr 9 | cos 9 | pat 24 | pad
AUX_EL = 2 * AUX_F32
W1_EL = 32 * 27 * 64               # [in=32][o=27][oc=64]
W2_EL = 64 * 27 * 128              # [in=64][o=27][oc=128]
W3_EL = 128 * 54 * 128             # [in=128][(o,h)=54][oc=128]
OFF_W1 = AUX_EL
OFF_W2 = OFF_W1 + W1_EL
OFF_W3 = OFF_W2 + W2_EL
BLOB_EL = OFF_W3 + W3_EL           # 1,164,640
SHARD_EL = BLOB_EL // NCORES       # 145,580

# conv1 tap groups: m=(a,b); partition block t in {0,1,2} holds tap (a,b,t)
C1_MS = [(a, b) for a in range(3) for b in range(3)]
# conv2 tap groups: (a, b, c0); block0 -> tap (a,b,c0), block1 -> (a,b,c0+1)
C2_MS = [(a, b, c0) for a in range(3) for b in range(3) for c0 in (0, 2)]


def _build_nc(n_cores=NCORES):
    nstat = 1.0 / (n_cores * S * 512)
    nc = bacc.Bacc("TRN2", target_bir_lowering=False, debug=False, num_devices=n_cores)

    pts_d = nc.dram_tensor("pts", [128, NCH, 3], F32, kind="ExternalInput")
    ctr_d = nc.dram_tensor("ctrv", [S * 3], F32, kind="ExternalInput")
    blob_d = nc.dram_tensor("blob", [SHARD_EL], BF16, kind="ExternalInput")
    desc_d = nc.dram_tensor("desc", [S, 256], F32, kind="ExternalOutput")

    with tile.TileContext(nc) as tc:
        with (
            tc.tile_pool(name="wp", bufs=1) as wp,
            tc.tile_pool(name="bigp", bufs=1) as bigp,
            tc.tile_pool(name="maskp", bufs=1) as maskp,
            tc.tile_pool(name="sampp", bufs=2) as sampp,
            tc.tile_pool(name="drainp", bufs=2) as drainp,
            tc.tile_pool(name="statp", bufs=1) as statp,
            tc.tile_pool(name="xrp", bufs=2) as xrp,
            tc.tile_pool(name="psb", bufs=2, space="PSUM") as psb,
            tc.tile_pool(name="psg", bufs=3, space="PSUM") as psg,
            tc.tile_pool(name="dramp", bufs=1, space="DRAM") as dramp,
        ):
            # ---------------- inputs + weight gather ----------------
            pts = wp.tile([128, NCH, 3], F32, tag="pts")
            ctrb = wp.tile([128, S, 3], F32, tag="ctrb")
            nc.sync.dma_start(pts[:], pts_d.ap())
            nc.sync.dma_start(
                ctrb[:],
                ctr_d.ap().rearrange("(s c) -> s c", s=S, c=3).unsqueeze(0).partition_broadcast(128))

            stg = dramp.tile([SHARD_EL], BF16, tag="stg")
            nc.sync.dma_start(stg[:], blob_d.ap())
            gath = dramp.tile([BLOB_EL], BF16, tag="gath")
            if n_cores > 1:
                nc.gpsimd.collective_compute(
                    "AllGather", OP.bypass,
                    replica_groups=[list(range(n_cores))],
                    ins=[stg.opt()], outs=[gath.opt()])
            else:
                nc.sync.dma_start(gath[:], stg[:])
            fv = gath[:].bitcast(F32)

            wr4 = wp.tile([3, 128], F32, tag="wr4")
            gb = wp.tile([128, 10], F32, tag="gb")
            thr = wp.tile([128, 9], F32, tag="thr")
            cosT = wp.tile([128, 9], F32, tag="cosT")
            pats = wp.tile([128, 3, 8], F32, tag="pats")
            nc.sync.dma_start(wr4[:], fv[0:384].rearrange("(p c) -> p c", p=3, c=128))
            nc.sync.dma_start(gb[:], fv[384:1664].rearrange("(p c) -> p c", p=128, c=10))
            nc.sync.dma_start(thr[:], fv[1664:1673].unsqueeze(0).partition_broadcast(128))
            nc.sync.dma_start(cosT[:], fv[1673:1682].unsqueeze(0).partition_broadcast(128))
            nc.sync.dma_start(
                pats[:],
                fv[1682:1706].rearrange("(g c) -> g c", g=3, c=8).unsqueeze(0).partition_broadcast(128))

            w1s = wp.tile([128, 9 * 128], BF16, tag="w1s")
            w2s = wp.tile([128, 18 * 128], BF16, tag="w2s")
            w3p = wp.tile([128, 54 * 128], BF16, tag="w3p")
            nc.vector.memset(w1s[:], 0.0)
            nc.vector.memset(w2s[:], 0.0)
            w1v = gath[OFF_W1:OFF_W2].rearrange("(p o f) -> p o f", p=32, o=27, f=64)
            for m, (a, b) in enumerate(C1_MS):
                for t in range(3):
                    o = a * 9 + b * 3 + t
                    for d in range(2):
                        nc.sync.dma_start(
                            w1s[32 * t:32 * (t + 1),
                                m * 128 + d * 64: m * 128 + (d + 1) * 64],
                            w1v[:, o, :])
            w2v = gath[OFF_W2:OFF_W3].rearrange("(p o f) -> p o f", p=64, o=27, f=128)
            for m, (a, b, c0) in enumerate(C2_MS):
                nc.sync.dma_start(
                    w2s[0:64, m * 128:(m + 1) * 128], w2v[:, a * 9 + b * 3 + c0, :])
                if c0 == 0:
                    nc.sync.dma_start(
                        w2s[64:128, m * 128:(m + 1) * 128], w2v[:, a * 9 + b * 3 + 1, :])
            nc.sync.dma_start(
                w3p[:], gath[OFF_W3:BLOB_EL].rearrange("(p c) -> p c", p=128, c=54 * 128))

            pad1 = bigp.tile([128, PADW], BF16, tag="pad1")
            pad2 = bigp.tile([128, PADW], BF16, tag="pad2")
            pad3 = bigp.tile([128, PADW], BF16, tag="pad3")
            nc.gpsimd.memset(pad1[:], 0.0)
            nc.gpsimd.memset(pad2[:], 0.0)
            nc.gpsimd.memset(pad3[:], 0.0)

            def box(pad, s0, ns, dd, dh, dw, p0=0, p1=128):
                base = dd * 100 + dh * 10 + dw
                v = pad[p0:p1, :].rearrange("p (s c) -> p s c", s=S, c=PD)
                v = v[:, s0:s0 + ns, base:base + 800]
                v = v.rearrange("p s (d x) -> p s d x", d=8)[:, :, :, 0:80]
                v = v.rearrange("p s d (h y) -> p s d h y", h=8)[:, :, :, :, 0:8]
                return v

            # ---------------- binning masks (whole-tile ops) ----------------
            rel = bigp.tile([128, S, NCH, 3], F32, tag="rel")
            nc.vector.tensor_tensor(
                rel[:],
                pts[:].unsqueeze(1).broadcast_to([128, S, NCH, 3]),
                ctrb[:].unsqueeze(2).broadcast_to([128, S, NCH, 3]),
                op=OP.subtract)
            relf = rel[:].rearrange("p s k c -> p (s k) c")
            xs_ = relf[:, :, 0]
            ys_ = relf[:, :, 1]
            zs_ = relf[:, :, 2]

            rho2 = maskp.tile([128, SK], F32, tag="rho2")
            tmp = maskp.tile([128, SK], F32, tag="tmp")
            nc.vector.tensor_tensor(rho2[:], xs_, xs_, op=OP.mult)
            nc.vector.tensor_tensor(tmp[:], ys_, ys_, op=OP.mult)
            nc.vector.tensor_tensor(rho2[:], rho2[:], tmp[:], op=OP.add)
            nc.vector.tensor_tensor(tmp[:], zs_, zs_, op=OP.mult)
            nc.vector.tensor_tensor(rho2[:], rho2[:], tmp[:], op=OP.add)
            rhoe = maskp.tile([128, SK], F32, tag="rhoe")
            nc.scalar.activation(rhoe[:], rho2[:], AF.Sqrt)

            ge9 = maskp.tile([128, SK, 9], BF16, tag="ge9")
            nc.vector.tensor_tensor(
                ge9[:],
                rho2[:].unsqueeze(2).broadcast_to([128, SK, 9]),
                thr[:].unsqueeze(1).broadcast_to([128, SK, 9]),
                op=OP.is_ge)
            oh_r = maskp.tile([128, SK, 8], BF16, tag="oh_r")
            nc.vector.tensor_tensor(oh_r[:], ge9[:, :, 0:8], ge9[:, :, 1:9], op=OP.subtract)

            gt9 = maskp.tile([128, SK, 9], BF16, tag="ge9")
            HSK = SK // 2
            for hh in range(2):
                rng2 = slice(hh * HSK, (hh + 1) * HSK)
                ct9 = maskp.tile([128, HSK, 9], F32, tag="ct9", name=f"ct9_{hh}")
                nc.vector.tensor_tensor(
                    ct9[:],
                    rhoe[:, rng2].unsqueeze(2).broadcast_to([128, HSK, 9]),
                    cosT[:].unsqueeze(1).broadcast_to([128, HSK, 9]),
                    op=OP.mult)
                nc.vector.tensor_tensor(
                    gt9[:, rng2, :], ct9[:],
                    zs_[:, rng2].unsqueeze(2).broadcast_to([128, HSK, 9]),
                    op=OP.is_gt)
            oh_t = maskp.tile([128, SK, 8], BF16, tag="oh_t")
            nc.vector.tensor_tensor(oh_t[:], gt9[:, :, 0:8], gt9[:, :, 1:9], op=OP.subtract)

            am = maskp.tile([128, SK], F32, tag="am")
            bm = maskp.tile([128, SK], F32, tag="bm")
            cm = maskp.tile([128, SK], F32, tag="cm")
            ax_ = maskp.tile([128, SK], F32, tag="ax")
            ay_ = maskp.tile([128, SK], F32, tag="ay")
            nc.vector.tensor_scalar(am[:], ys_, 0.0, None, op0=OP.is_ge)
            nc.vector.tensor_scalar(bm[:], xs_, 0.0, None, op0=OP.is_ge)
            nc.scalar.activation(ax_[:], xs_, AF.Abs)
            nc.scalar.activation(ay_[:], ys_, AF.Abs)
            nc.vector.tensor_tensor(cm[:], ay_[:], ax_[:], op=OP.is_ge)
            da = maskp.tile([128, SK, 8], BF16, tag="da")
            db = maskp.tile([128, SK, 8], BF16, tag="db")
            oh_p = maskp.tile([128, SK, 8], BF16, tag="oh_p")
            nc.vector.tensor_tensor(
                da[:], am[:].unsqueeze(2).broadcast_to([128, SK, 8]),
                pats[:, 0, :].unsqueeze(1).broadcast_to([128, SK, 8]), op=OP.is_equal)
            nc.vector.tensor_tensor(
                db[:], bm[:].unsqueeze(2).broadcast_to([128, SK, 8]),
                pats[:, 1, :].unsqueeze(1).broadcast_to([128, SK, 8]), op=OP.is_equal)
            nc.vector.tensor_tensor(
                oh_p[:], cm[:].unsqueeze(2).broadcast_to([128, SK, 8]),
                pats[:, 2, :].unsqueeze(1).broadcast_to([128, SK, 8]), op=OP.is_equal)
            nc.vector.tensor_tensor(da[:], da[:], db[:], op=OP.mult)
            nc.vector.tensor_tensor(oh_p[:], oh_p[:], da[:], op=OP.mult)

            # ---------------- binning matmuls (f32, groups of 2 centers) ---
            sums = statp.tile([64, S, 4, 8], F32, tag="sums")
            for gp in range(4):
                tiles = []
                for gl in range(2):
                    g = 2 * gp + gl
                    rng = slice(g * 32, (g + 1) * 32)
                    ohrt = sampp.tile([128, 32, 8, 8], F32, tag="ohrt", name=f"ohrt{g}")
                    nc.vector.tensor_tensor(
                        ohrt[:],
                        oh_r[:, rng, :].unsqueeze(3).broadcast_to([128, 32, 8, 8]),
                        oh_t[:, rng, :].unsqueeze(2).broadcast_to([128, 32, 8, 8]),
                        op=OP.mult)
                    wp4 = sampp.tile([128, 32, 4, 8], F32, tag="wp4", name=f"wp4{g}")
                    nc.vector.tensor_tensor(
                        wp4[:, :, 0:3, :],
                        relf[:, rng, :].unsqueeze(3).broadcast_to([128, 32, 3, 8]),
                        oh_p[:, rng, :].unsqueeze(2).broadcast_to([128, 32, 3, 8]),
                        op=OP.mult)
                    nc.vector.tensor_copy(wp4[:, :, 3, :], oh_p[:, rng, :])
                    ps = psb.tile([64, 2, 32], F32, tag="binp", name=f"binps{g}")
                    tiles.append((ohrt, wp4, ps))
                # interleave chains across the two PSUM tiles (banks) only;
                # same-bank interleave of f32 self-loading chains corrupts
                for sloc in range(2):
                    for k in range(NCH):
                        for gl in range(2):
                            ohrt, wp4, ps = tiles[gl]
                            nc.tensor.matmul(ps[:, sloc, :],
                                             ohrt[:, sloc * NCH + k],
                                             wp4[:, sloc * NCH + k],
                                             start=(k == 0), stop=(k == NCH - 1))
                for gl in range(2):
                    g = 2 * gp + gl
                    nc.vector.tensor_copy(
                        sums[:, 2 * g:2 * (g + 1), :, :],
                        tiles[gl][2][:].rearrange("p s (e f) -> p s e f", e=4))

            cnt = sums[:, :, 3, :]
            nc.vector.tensor_scalar(cnt, cnt, 1.0, None, op0=OP.max)
            nc.vector.reciprocal(cnt, cnt)
            nc.vector.tensor_tensor(
                sums[:, :, 0:3, :], sums[:, :, 0:3, :],
                sums[:, :, 3:4, :].broadcast_to([64, S, 3, 8]), op=OP.mult)

            binf_d = dramp.tile([3, S * 512], F32, tag="binfd")
            for c in range(3):
                nc.sync.dma_start(
                    binf_d[c].rearrange("(s rt p) -> rt s p", s=S, rt=64, p=8),
                    sums[:, :, c, :])

            # ---------------- raising (f32 matmul) + BN0 stats -------------
            pt0 = statp.tile([128, 16], F32, tag="pt0")
            qt0 = statp.tile([128, 8], F32, tag="qt0")
            nc.vector.memset(pt0[32:64, :], 0.0)
            nc.vector.memset(pt0[64:128, :], 0.0)
            for pr in range(8):
                xr = xrp.tile([3, 2, 512], F32, tag="xr", name=f"xr{pr}")
                nc.sync.dma_start(
                    xr[:], binf_d[:, pr * 1024:(pr + 1) * 1024]
                    .rearrange("p (s v) -> p s v", s=2))
                ps = psg.tile([128, 1024], F32, tag="big", name=f"rps{pr}")
                for sloc in range(2):
                    nc.tensor.matmul(ps[:, sloc * 512:(sloc + 1) * 512],
                                     wr4[:], xr[:, sloc], start=True, stop=True)
                for sloc in range(2):
                    s = 2 * pr + sloc
                    nc.scalar.activation(
                        box(pad1, s, 1, 1, 1, 1, p0=0, p1=32)[:, 0],
                        ps[0:32, sloc * 512:(sloc + 1) * 512].rearrange(
                            "p (d h w) -> p d h w", d=8, h=8),
                        AF.Copy, accum_out=pt0[0:32, s:s + 1])
                sqt = drainp.tile([128, 1024], BF16, tag="sqt", name=f"sq0_{pr}")
                nc.scalar.activation(sqt[:], ps[:], AF.Square,
                                     accum_out=qt0[:, pr:pr + 1])

            epst = statp.tile([128, 1], F32, tag="epst")
            nc.vector.memset(epst[:], EPS)

            def bn_multi(specs, layer):
                # specs: list of (ptile, qtile, gcol, bcol, stride, off);
                # one AllReduce carries all groups' (sum, sumsq) stats
                ng = len(specs)
                st = statp.tile([128, 2 * ng], F32, tag=f"st{layer}")
                for i, (ptile, qtile, _, _, stride, off) in enumerate(specs):
                    for j, tl in ((0, ptile), (1, qtile)):
                        src = tl[:]
                        if stride != 1:
                            src = tl[:].rearrange(
                                "p (s h) -> p s h", h=stride)[:, :, off]
                        nc.vector.tensor_reduce(
                            st[:, 2 * i + j:2 * i + j + 1], src, axis=AX.X, op=OP.add)
                cc_in = dramp.tile([128, 2 * ng], F32, tag=f"cci{layer}")
                cc_out = dramp.tile([128, 2 * ng], F32, tag=f"cco{layer}")
                nc.sync.dma_start(cc_in[:], st[:])
                if n_cores > 1:
                    nc.gpsimd.collective_compute(
                        "AllReduce", OP.add,
                        replica_groups=[list(range(n_cores))],
                        ins=[cc_in.opt()], outs=[cc_out.opt()])
                else:
                    nc.sync.dma_start(cc_out[:], cc_in[:])
                gst = statp.tile([128, 2 * ng], F32, tag=f"gst{layer}")
                nc.sync.dma_start(gst[:], cc_out[:])
                out = []
                for i, (_, _, gcol, bcol, _, off) in enumerate(specs):
                    mean = statp.tile([128, 1], F32, tag=f"mean{layer}_{i}")
                    var = statp.tile([128, 1], F32, tag=f"var{layer}_{i}")
                    sc = statp.tile([128, 1], F32, tag=f"sc{layer}_{i}")
                    bi = statp.tile([128, 1], F32, tag=f"bi{layer}_{i}")
                    nc.vector.tensor_scalar(mean[:], gst[:, 2 * i:2 * i + 1],
                                            nstat, None, op0=OP.mult)
                    nc.vector.tensor_tensor(var[:], mean[:], mean[:], op=OP.mult)
                    nc.vector.scalar_tensor_tensor(
                        var[:], gst[:, 2 * i + 1:2 * i + 2], nstat, var[:],
                        op0=OP.mult, op1=OP.subtract)
                    nc.scalar.activation(var[:], var[:], AF.Sqrt, bias=epst[:])
                    nc.vector.reciprocal(var[:], var[:])
                    nc.vector.tensor_tensor(sc[:], gb[:, gcol:gcol + 1], var[:], op=OP.mult)
                    nc.vector.tensor_tensor(bi[:], mean[:], sc[:], op=OP.mult)
                    nc.vector.tensor_tensor(bi[:], gb[:, bcol:bcol + 1], bi[:],
                                            op=OP.subtract)
                    out.append((sc, bi))
                return out

            def bn_block(ptile, qtile, gcol, bcol, layer):
                return bn_multi([(ptile, qtile, gcol, bcol, 1, 0)], layer)[0]

            def relu_pass(pad, sc, bi, bsz):
                # y = relu(sc*x + bi) on every center's box, block 0 only;
                # sigma-shifted copies are made afterwards by shift-DMAs
                for s in range(S):
                    v = box(pad, s, 1, 1, 1, 1, p0=0, p1=bsz)[:, 0]
                    nc.scalar.activation(v, v, AF.Relu,
                                         bias=bi[0:bsz], scale=sc[0:bsz])

            sc0, bi0 = bn_block(pt0, qt0, 0, 1, 0)
            relu_pass(pad1, sc0, bi0, 32)
            for t in (1, 2):
                nc.sync.dma_start(pad1[32 * t:32 * (t + 1), 0:PADW - t],
                                  pad1[0:32, t:PADW])

            # ---------------- conv1 (sigma-packed, 9 matmuls/center) -------
            pt1 = statp.tile([128, 16], F32, tag="pt1")
            qt1 = statp.tile([128, 8], F32, tag="qt1")
            nc.vector.memset(pt1[64:128, :], 0.0)
            for prp in range(4):
                pss = [psg.tile([128, 1024], F32, tag="big", name=f"c1ps{2 * prp + i}")
                       for i in range(2)]
                for m, (a, b) in enumerate(C1_MS):
                    lhsT = w1s[:, m * 128:(m + 1) * 128]
                    for i in range(2):
                        for sloc in range(2):
                            rhs = box(pad1, 2 * (2 * prp + i) + sloc, 1, a, b, 0)[:, 0]
                            nc.tensor.matmul(
                                pss[i][:, sloc * 512:(sloc + 1) * 512],
                                lhsT, rhs, start=(m == 0), stop=(m == 8))
                for i in range(2):
                    pr = 2 * prp + i
                    for sloc in range(2):
                        s = 2 * pr + sloc
                        nc.scalar.activation(
                            box(pad2, s, 1, 1, 1, 1, p0=0, p1=64)[:, 0],
                            pss[i][0:64, sloc * 512:(sloc + 1) * 512].rearrange(
                                "p (d h w) -> p d h w", d=8, h=8),
                            AF.Copy, accum_out=pt1[0:64, s:s + 1])
                    sqt = drainp.tile([128, 1024], BF16, tag="sqt", name=f"sq1_{pr}")
                    nc.scalar.activation(sqt[:], pss[i][:], AF.Square,
                                         accum_out=qt1[:, pr:pr + 1])

            sc1, bi1 = bn_block(pt1, qt1, 2, 3, 1)
            relu_pass(pad2, sc1, bi1, 64)
            nc.sync.dma_start(pad2[64:128, 0:PADW - 1], pad2[0:64, 1:PADW])

            # ---------------- conv2 (sigma-packed, 18 matmuls/center) ------
            pt2 = statp.tile([128, 16], F32, tag="pt2")
            qt2 = statp.tile([128, 8], F32, tag="qt2")
            for prp in range(4):
                pss = [psg.tile([128, 1024], F32, tag="big", name=f"c2ps{2 * prp + i}")
                       for i in range(2)]
                for m, (a, b, c0) in enumerate(C2_MS):
                    lhsT = w2s[:, m * 128:(m + 1) * 128]
                    for i in range(2):
                        for sloc in range(2):
                            rhs = box(pad2, 2 * (2 * prp + i) + sloc, 1, a, b, c0)[:, 0]
                            nc.tensor.matmul(
                                pss[i][:, sloc * 512:(sloc + 1) * 512],
                                lhsT, rhs, start=(m == 0), stop=(m == 17))
                for i in range(2):
                    pr = 2 * prp + i
                    for sloc in range(2):
                        s = 2 * pr + sloc
                        nc.scalar.activation(
                            box(pad3, s, 1, 1, 1, 1)[:, 0],
                            pss[i][:, sloc * 512:(sloc + 1) * 512].rearrange(
                                "p (d h w) -> p d h w", d=8, h=8),
                            AF.Copy, accum_out=pt2[:, s:s + 1])
                    sqt = drainp.tile([128, 1024], BF16, tag="sqt", name=f"sq2_{pr}")
                    nc.scalar.activation(sqt[:], pss[i][:], AF.Square,
                                         accum_out=qt2[:, pr:pr + 1])

            sc2, bi2 = bn_block(pt2, qt2, 4, 5, 2)
            relu_pass(pad3, sc2, bi2, 128)

            # ---------------- conv3 (27 taps x 2 halves per center) --------
            pt3 = statp.tile([128, 2 * S], F32, tag="pt3")
            qt3 = statp.tile([128, 2 * S], F32, tag="qt3")
            mxs = statp.tile([128, 2, S], F32, tag="mxs")
            mns = statp.tile([128, 2, S], F32, tag="mns")
            for sp in range(8):
                pss = [psg.tile([128, 1024], F32, tag="big", name=f"c3ps{2 * sp + i}")
                       for i in range(2)]
                for o in range(27):
                    dd, dh, dw = o // 9, (o // 3) % 3, o % 3
                    for h in range(2):
                        lhsT = w3p[:, (o * 2 + h) * 128:(o * 2 + h + 1) * 128]
                        for i in range(2):
                            rhs = box(pad3, 2 * sp + i, 1, dd, dh, dw)[:, 0]
                            nc.tensor.matmul(pss[i][:, h * 512:(h + 1) * 512],
                                             lhsT, rhs, start=(o == 0), stop=(o == 26))
                for i in range(2):
                    s = 2 * sp + i
                    nc.vector.tensor_reduce(
                        mxs[:, :, s:s + 1],
                        pss[i][:].rearrange("p (h v) -> p h v", h=2),
                        axis=AX.X, op=OP.max)
                    nc.vector.tensor_reduce(
                        mns[:, :, s:s + 1],
                        pss[i][:].rearrange("p (h v) -> p h v", h=2),
                        axis=AX.X, op=OP.min)
                    for h in range(2):
                        sqa = drainp.tile([128, 512], BF16, tag="sqf", name=f"sqf_{s}_{h}")
                        nc.scalar.activation(sqa[:], pss[i][:, h * 512:(h + 1) * 512],
                                             AF.Copy,
                                             accum_out=pt3[:, s * 2 + h:s * 2 + h + 1])
                        sqb = drainp.tile([128, 512], BF16, tag="s2q", name=f"s2q_{s}_{h}")
                        nc.scalar.activation(sqb[:], pss[i][:, h * 512:(h + 1) * 512],
                                             AF.Square,
                                             accum_out=qt3[:, s * 2 + h:s * 2 + h + 1])

            desc_sb = statp.tile([128, 2, S], F32, tag="descsb")
            bn3 = bn_multi([(pt3, qt3, 6, 7, 2, 0), (pt3, qt3, 8, 9, 2, 1)], 3)
            for h in range(2):
                sc3, bi3 = bn3[h]
                zmx = statp.tile([128, S], F32, tag=f"zmx{h}")
                zmn = statp.tile([128, S], F32, tag=f"zmn{h}")
                nc.vector.tensor_scalar(zmx[:], mxs[:, h, :], sc3[:], bi3[:],
                                        op0=OP.mult, op1=OP.add)
                nc.vector.tensor_scalar(zmn[:], mns[:, h, :], sc3[:], bi3[:],
                                        op0=OP.mult, op1=OP.add)
                csel = statp.tile([128, 1], F32, tag=f"csel{h}")
                nc.vector.tensor_scalar(csel[:], sc3[:], 0.0, None, op0=OP.is_ge)
                nc.vector.tensor_tensor(zmx[:], zmx[:], zmn[:], op=OP.subtract)
                nc.vector.scalar_tensor_tensor(zmx[:], zmx[:], csel[:], zmn[:],
                                               op0=OP.mult, op1=OP.add)
                nc.vector.tensor_scalar(desc_sb[:, h, :], zmx[:], 0.0, None, op0=OP.max)

            for h in range(2):
                nc.sync.dma_start(
                    desc_d.ap().rearrange("s (h ch) -> ch h s", h=2)[:, h, :],
                    desc_sb[:, h, :])

    nc.compile()
    return nc


def _host_pack(inputs):
    pts_all = np.asarray(inputs["points"], np.float32)
    ctr_all = np.asarray(inputs["center_points"], np.float32)
    w_raise = np.asarray(inputs["w_raise"], np.float32)
    w1 = np.asarray(inputs["w1"], np.float32)
    w2 = np.asarray(inputs["w2"], np.float32)
    w3 = np.asarray(inputs["w3"], np.float32)

    wr4 = np.zeros((3, 128), np.float32)
    for g in range(4):
        wr4[:, g * 32:(g + 1) * 32] = w_raise.T
    gb = np.zeros((128, 10), np.float32)
    g0 = np.asarray(inputs["g0"], np.float32); be0 = np.asarray(inputs["be0"], np.float32)
    g1 = np.asarray(inputs["g1"], np.float32); be1 = np.asarray(inputs["be1"], np.float32)
    g2 = np.asarray(inputs["g2"], np.float32); be2 = np.asarray(inputs["be2"], np.float32)
    g3 = np.asarray(inputs["g3"], np.float32); be3 = np.asarray(inputs["be3"], np.float32)
    gb[:, 0] = np.tile(g0, 4); gb[:, 1] = np.tile(be0, 4)
    gb[:, 2] = np.tile(g1, 2); gb[:, 3] = np.tile(be1, 2)
    gb[:, 4] = g2; gb[:, 5] = be2
    gb[:, 6] = g3[:128]; gb[:, 7] = be3[:128]
    gb[:, 8] = g3[128:]; gb[:, 9] = be3[128:]

    aux = np.zeros(AUX_F32, np.float32)
    aux[0:384] = wr4.reshape(-1)
    aux[384:1664] = gb.reshape(-1)
    aux[1664:1673] = [-1e30] + R2_B + [3e38]
    aux[1673:1682] = [4.0] + COS_B + [-4.0]
    aux[1682:1690] = [0, 0, 0, 0, 1, 1, 1, 1]
    aux[1690:1698] = [0, 0, 1, 1, 1, 1, 0, 0]
    aux[1698:1706] = [0, 1, 1, 0, 0, 1, 1, 0]

    blob = np.empty(BLOB_EL, ml_dtypes.bfloat16)
    blob[0:AUX_EL] = aux.view(ml_dtypes.bfloat16)
    blob[OFF_W1:OFF_W2] = np.ascontiguousarray(
        w1.reshape(64, 32, 27).transpose(1, 2, 0)).astype(ml_dtypes.bfloat16).reshape(-1)
    blob[OFF_W2:OFF_W3] = np.ascontiguousarray(
        w2.reshape(128, 64, 27).transpose(1, 2, 0)).astype(ml_dtypes.bfloat16).reshape(-1)
    blob[OFF_W3:BLOB_EL] = np.ascontiguousarray(
        w3.reshape(2, 128, 128, 27).transpose(2, 3, 0, 1)).astype(
            ml_dtypes.bfloat16).reshape(-1)

    in_maps = []
    for c in range(NCORES):
        b = c // 4
        pts = np.ascontiguousarray(np.transpose(pts_all[b].reshape(NCH, 128, 3), (1, 0, 2)))
        s0 = (c % 4) * S
        ctrv = np.ascontiguousarray(ctr_all[b, s0:s0 + S, :].reshape(-1))
        shard = np.ascontiguousarray(blob[c * SHARD_EL:(c + 1) * SHARD_EL])
        in_maps.append(dict(pts=pts, ctrv=ctrv, blob=shard))
    return in_maps


_CACHED_NC = None


def _get_nc():
    global _CACHED_NC
    if _CACHED_NC is None:
        _CACHED_NC = _build_nc(NCORES)
    return _CACHED_NC


def kernel(**inputs) -> np.ndarray:
    nc = _get_nc()
    in_maps = _host_pack(inputs)
    res = bass_utils.run_bass_kernel_spmd(nc, in_maps, core_ids=list(range(NCORES)))
    out = np.concatenate([np.asarray(res.results[c]["desc"], np.float32)
                          for c in range(NCORES)], axis=0)     # (128, 256)
    return out.reshape(B, M, 256)


# revision 30
# speedup vs baseline: 1.0255x; 1.0255x over previous
"""Self-contained MiniSpinNet kernel for 8 Trainium2 NeuronCores.

kernel(**inputs) takes the FULL unsharded inputs (as produced by
setup_inputs()) and returns the full (2, 64, 256) float32 descriptor.

Data-parallel over the 128 B*M centers (16 per core).  The environment
charges a large flat cost per *instruction* (~80us/matmul, ~45us/vector
op), so the kernel is structured to minimize instruction count:
  - weights are shipped sharded 1/8-per-core and AllGathered on device
  - binning matmuls run in f32 (no hi/lo split, single instruction each)
  - conv1/conv2 pack 3/2 kernel taps into the 128-row contraction via
    column-shifted copies of the activations (sigma-shift packing)
  - mask building and drains are merged into wide single instructions
"""
import numpy as np
import ml_dtypes

import concourse.bass as bass
import concourse.bacc as bacc
import concourse.mybir as mybir
import concourse.tile as tile
from concourse import bass_utils

F32 = mybir.dt.float32
BF16 = mybir.dt.bfloat16
AF = mybir.ActivationFunctionType
OP = mybir.AluOpType
AX = mybir.AxisListType

B, N, M = 2, 2048, 64
BM = B * M
S = 16
NCORES = 8
NCH = 16
SK = S * NCH
EPS = 1e-5

COS_B = [float(np.float32(np.cos(j * np.pi / 8))) for j in range(1, 8)]
R2_B = [float(np.float32((j / 16.0) ** 2)) for j in range(1, 8)]

PD = 1024
PADW = S * PD

# blob layout (bf16 elements): aux f32 pairs, then compact bf16 weights
AUX_F32 = 1712                     # wr4 384 | gb 1280 | thr 9 | cos 9 | pat 24 | pad
AUX_EL = 2 * AUX_F32
W1_EL = 32 * 27 * 64               # [in=32][o=27][oc=64]
W2_EL = 64 * 27 * 128              # [in=64][o=27][oc=128]
W3_EL = 128 * 54 * 128             # [in=128][(o,h)=54][oc=128]
OFF_W1 = AUX_EL
OFF_W2 = OFF_W1 + W1_EL
OFF_W3 = OFF_W2 + W2_EL
BLOB_EL = OFF_W3 + W3_EL           # 1,164,640
SHARD_EL = BLOB_EL // NCORES       # 145,580

# conv1 tap groups: m=(a,b); partition block t in {0,1,2} holds tap (a,b,t)
C1_MS = [(a, b) for a in range(3) for b in range(3)]
# conv2 tap groups: (a, b, c0); block0 -> tap (a,b,c0), block1 -> (a,b,c0+1)
C2_MS = [(a, b, c0) for a in range(3) for b in range(3) for c0 in (0, 2)]


def _build_nc(n_cores=NCORES):
    nstat = 1.0 / (n_cores * S * 512)
    nc = bacc.Bacc("TRN2", target_bir_lowering=False, debug=False, num_devices=n_cores)

    pts_d = nc.dram_tensor("pts", [128, NCH, 3], F32, kind="ExternalInput")
    ctr_d = nc.dram_tensor("ctrv", [S * 3], F32, kind="ExternalInput")
    blob_d = nc.dram_tensor("blob", [SHARD_EL], BF16, kind="ExternalInput")
    desc_d = nc.dram_tensor("desc", [S, 256], F32, kind="ExternalOutput")

    with tile.TileContext(nc) as tc:
        with (
            tc.tile_pool(name="wp", bufs=1) as wp,
            tc.tile_pool(name="bigp", bufs=1) as bigp,
            tc.tile_pool(name="maskp", bufs=1) as maskp,
            tc.tile_pool(name="sampp", bufs=2) as sampp,
            tc.tile_pool(name="drainp", bufs=2) as drainp,
            tc.tile_pool(name="statp", bufs=1) as statp,
            tc.tile_pool(name="xrp", bufs=2) as xrp,
            tc.tile_pool(name="psb", bufs=2, space="PSUM") as psb,
            tc.tile_pool(name="psg", bufs=3, space="PSUM") as psg,
            tc.tile_pool(name="dramp", bufs=1, space="DRAM") as dramp,
        ):
            # ---------------- inputs + weight gather ----------------
            pts = wp.tile([128, NCH, 3], F32, tag="pts")
            ctrb = wp.tile([128, S, 3], F32, tag="ctrb")
            nc.sync.dma_start(pts[:], pts_d.ap())
            nc.sync.dma_start(
                ctrb[:],
                ctr_d.ap().rearrange("(s c) -> s c", s=S, c=3).unsqueeze(0).partition_broadcast(128))

            stg = dramp.tile([SHARD_EL], BF16, tag="stg")
            nc.sync.dma_start(stg[:], blob_d.ap())
            gath = dramp.tile([BLOB_EL], BF16, tag="gath")
            if n_cores > 1:
                nc.gpsimd.collective_compute(
                    "AllGather", OP.bypass,
                    replica_groups=[list(range(n_cores))],
                    ins=[stg.opt()], outs=[gath.opt()])
            else:
                nc.sync.dma_start(gath[:], stg[:])
            fv = gath[:].bitcast(F32)

            wr4 = wp.tile([3, 128], F32, tag="wr4")
            gb = wp.tile([128, 10], F32, tag="gb")
            thr = wp.tile([128, 9], F32, tag="thr")
            cosT = wp.tile([128, 9], F32, tag="cosT")
            pats = wp.tile([128, 3, 8], F32, tag="pats")
            nc.sync.dma_start(wr4[:], fv[0:384].rearrange("(p c) -> p c", p=3, c=128))
            nc.sync.dma_start(gb[:], fv[384:1664].rearrange("(p c) -> p c", p=128, c=10))
            nc.sync.dma_start(thr[:], fv[1664:1673].unsqueeze(0).partition_broadcast(128))
            nc.sync.dma_start(cosT[:], fv[1673:1682].unsqueeze(0).partition_broadcast(128))
            nc.sync.dma_start(
                pats[:],
                fv[1682:1706].rearrange("(g c) -> g c", g=3, c=8).unsqueeze(0).partition_broadcast(128))

            w1s = wp.tile([128, 9 * 128], BF16, tag="w1s")
            w2s = wp.tile([128, 18 * 128], BF16, tag="w2s")
            w3p = wp.tile([128, 54 * 128], BF16, tag="w3p")
            nc.vector.memset(w1s[:], 0.0)
            nc.vector.memset(w2s[:], 0.0)
            w1v = gath[OFF_W1:OFF_W2].rearrange("(p o f) -> p o f", p=32, o=27, f=64)
            for m, (a, b) in enumerate(C1_MS):
                for t in range(3):
                    o = a * 9 + b * 3 + t
                    for d in range(2):
                        nc.sync.dma_start(
                            w1s[32 * t:32 * (t + 1),
                                m * 128 + d * 64: m * 128 + (d + 1) * 64],
                            w1v[:, o, :])
            w2v = gath[OFF_W2:OFF_W3].rearrange("(p o f) -> p o f", p=64, o=27, f=128)
            for m, (a, b, c0) in enumerate(C2_MS):
                nc.sync.dma_start(
                    w2s[0:64, m * 128:(m + 1) * 128], w2v[:, a * 9 + b * 3 + c0, :])
                if c0 == 0:
                    nc.sync.dma_start(
                        w2s[64:128, m * 128:(m + 1) * 128], w2v[:, a * 9 + b * 3 + 1, :])
            nc.sync.dma_start(
                w3p[:], gath[OFF_W3:BLOB_EL].rearrange("(p c) -> p c", p=128, c=54 * 128))

            pad1 = bigp.tile([128, PADW], BF16, tag="pad1")
            pad2 = bigp.tile([128, PADW], BF16, tag="pad2")
            pad3 = bigp.tile([128, PADW], BF16, tag="pad3")
            nc.gpsimd.memset(pad1[:], 0.0)
            nc.gpsimd.memset(pad2[:], 0.0)
            nc.gpsimd.memset(pad3[:], 0.0)

            def box(pad, s0, ns, dd, dh, dw, p0=0, p1=128):
                base = dd * 100 + dh * 10 + dw
                v = pad[p0:p1, :].rearrange("p (s c) -> p s c", s=S, c=PD)
                v = v[:, s0:s0 + ns, base:base + 800]
                v = v.rearrange("p s (d x) -> p s d x", d=8)[:, :, :, 0:80]
                v = v.rearrange("p s d (h y) -> p s d h y", h=8)[:, :, :, :, 0:8]
                return v

            # ---------------- binning masks (whole-tile ops) ----------------
            rel = bigp.tile([128, S, NCH, 3], F32, tag="rel")
            nc.vector.tensor_tensor(
                rel[:],
                pts[:].unsqueeze(1).broadcast_to([128, S, NCH, 3]),
                ctrb[:].unsqueeze(2).broadcast_to([128, S, NCH, 3]),
                op=OP.subtract)
            relf = rel[:].rearrange("p s k c -> p (s k) c")
            xs_ = relf[:, :, 0]
            ys_ = relf[:, :, 1]
            zs_ = relf[:, :, 2]

            rho2 = maskp.tile([128, SK], F32, tag="rho2")
            tmp = maskp.tile([128, SK], F32, tag="tmp")
            nc.vector.tensor_tensor(rho2[:], xs_, xs_, op=OP.mult)
            nc.vector.tensor_tensor(tmp[:], ys_, ys_, op=OP.mult)
            nc.vector.tensor_tensor(rho2[:], rho2[:], tmp[:], op=OP.add)
            nc.vector.tensor_tensor(tmp[:], zs_, zs_, op=OP.mult)
            nc.vector.tensor_tensor(rho2[:], rho2[:], tmp[:], op=OP.add)
            rhoe = maskp.tile([128, SK], F32, tag="rhoe")
            nc.scalar.activation(rhoe[:], rho2[:], AF.Sqrt)

            ge9 = maskp.tile([128, SK, 9], BF16, tag="ge9")
            nc.vector.tensor_tensor(
                ge9[:],
                rho2[:].unsqueeze(2).broadcast_to([128, SK, 9]),
                thr[:].unsqueeze(1).broadcast_to([128, SK, 9]),
                op=OP.is_ge)
            oh_r = maskp.tile([128, SK, 8], BF16, tag="oh_r")
            nc.vector.tensor_tensor(oh_r[:], ge9[:, :, 0:8], ge9[:, :, 1:9], op=OP.subtract)

            gt9 = maskp.tile([128, SK, 9], BF16, tag="ge9")
            HSK = SK // 2
            for hh in range(2):
                rng2 = slice(hh * HSK, (hh + 1) * HSK)
                ct9 = maskp.tile([128, HSK, 9], F32, tag="ct9", name=f"ct9_{hh}")
                nc.vector.tensor_tensor(
                    ct9[:],
                    rhoe[:, rng2].unsqueeze(2).broadcast_to([128, HSK, 9]),
                    cosT[:].unsqueeze(1).broadcast_to([128, HSK, 9]),
                    op=OP.mult)
                nc.vector.tensor_tensor(
                    gt9[:, rng2, :], ct9[:],
                    zs_[:, rng2].unsqueeze(2).broadcast_to([128, HSK, 9]),
                    op=OP.is_gt)
            oh_t = maskp.tile([128, SK, 8], BF16, tag="oh_t")
            nc.vector.tensor_tensor(oh_t[:], gt9[:, :, 0:8], gt9[:, :, 1:9], op=OP.subtract)

            am = maskp.tile([128, SK], F32, tag="am")
            bm = maskp.tile([128, SK], F32, tag="bm")
            cm = maskp.tile([128, SK], F32, tag="cm")
            ax_ = maskp.tile([128, SK], F32, tag="ax")
            ay_ = maskp.tile([128, SK], F32, tag="ay")
            nc.vector.tensor_scalar(am[:], ys_, 0.0, None, op0=OP.is_ge)
            nc.vector.tensor_scalar(bm[:], xs_, 0.0, None, op0=OP.is_ge)
            nc.scalar.activation(ax_[:], xs_, AF.Abs)
            nc.scalar.activation(ay_[:], ys_, AF.Abs)
            nc.vector.tensor_tensor(cm[:], ay_[:], ax_[:], op=OP.is_ge)
            da = maskp.tile([128, SK, 8], BF16, tag="da")
            db = maskp.tile([128, SK, 8], BF16, tag="db")
            oh_p = maskp.tile([128, SK, 8], BF16, tag="oh_p")
            nc.vector.tensor_tensor(
                da[:], am[:].unsqueeze(2).broadcast_to([128, SK, 8]),
                pats[:, 0, :].unsqueeze(1).broadcast_to([128, SK, 8]), op=OP.is_equal)
            nc.vector.tensor_tensor(
                db[:], bm[:].unsqueeze(2).broadcast_to([128, SK, 8]),
                pats[:, 1, :].unsqueeze(1).broadcast_to([128, SK, 8]), op=OP.is_equal)
            nc.vector.tensor_tensor(
                oh_p[:], cm[:].unsqueeze(2).broadcast_to([128, SK, 8]),
                pats[:, 2, :].unsqueeze(1).broadcast_to([128, SK, 8]), op=OP.is_equal)
            nc.vector.tensor_tensor(da[:], da[:], db[:], op=OP.mult)
            nc.vector.tensor_tensor(oh_p[:], oh_p[:], da[:], op=OP.mult)

            # ---------------- binning matmuls (f32, groups of 2 centers) ---
            sums = statp.tile([64, S, 4, 8], F32, tag="sums")
            for gp in range(4):
                tiles = []
                for gl in range(2):
                    g = 2 * gp + gl
                    rng = slice(g * 32, (g + 1) * 32)
                    ohrt = sampp.tile([128, 32, 8, 8], F32, tag="ohrt", name=f"ohrt{g}")
                    nc.vector.tensor_tensor(
                        ohrt[:],
                        oh_r[:, rng, :].unsqueeze(3).broadcast_to([128, 32, 8, 8]),
                        oh_t[:, rng, :].unsqueeze(2).broadcast_to([128, 32, 8, 8]),
                        op=OP.mult)
                    wp4 = sampp.tile([128, 32, 4, 8], F32, tag="wp4", name=f"wp4{g}")
                    nc.vector.tensor_tensor(
                        wp4[:, :, 0:3, :],
                        relf[:, rng, :].unsqueeze(3).broadcast_to([128, 32, 3, 8]),
                        oh_p[:, rng, :].unsqueeze(2).broadcast_to([128, 32, 3, 8]),
                        op=OP.mult)
                    nc.vector.tensor_copy(wp4[:, :, 3, :], oh_p[:, rng, :])
                    ps = psb.tile([64, 2, 32], F32, tag="binp", name=f"binps{g}")
                    tiles.append((ohrt, wp4, ps))
                # serial per chain: f32 self-loading matmul chains may not be
                # interleaved within a PSUM bank (corrupts accumulation), and
                # cross-bank interleave measured no faster than serial
                for gl in range(2):
                    ohrt, wp4, ps = tiles[gl]
                    for sloc in range(2):
                        for k in range(NCH):
                            nc.tensor.matmul(ps[:, sloc, :],
                                             ohrt[:, sloc * NCH + k],
                                             wp4[:, sloc * NCH + k],
                                             start=(k == 0), stop=(k == NCH - 1))
                for gl in range(2):
                    g = 2 * gp + gl
                    nc.vector.tensor_copy(
                        sums[:, 2 * g:2 * (g + 1), :, :],
                        tiles[gl][2][:].rearrange("p s (e f) -> p s e f", e=4))

            cnt = sums[:, :, 3, :]
            nc.vector.tensor_scalar(cnt, cnt, 1.0, None, op0=OP.max)
            nc.vector.reciprocal(cnt, cnt)
            nc.vector.tensor_tensor(
                sums[:, :, 0:3, :], sums[:, :, 0:3, :],
                sums[:, :, 3:4, :].broadcast_to([64, S, 3, 8]), op=OP.mult)

            binf_d = dramp.tile([3, S * 512], F32, tag="binfd")
            for c in range(3):
                nc.sync.dma_start(
                    binf_d[c].rearrange("(s rt p) -> rt s p", s=S, rt=64, p=8),
                    sums[:, :, c, :])

            # ---------------- raising (f32 matmul) + BN0 stats -------------
            pt0 = statp.tile([128, 16], F32, tag="pt0")
            qt0 = statp.tile([128, 8], F32, tag="qt0")
            nc.vector.memset(pt0[32:64, :], 0.0)
            nc.vector.memset(pt0[64:128, :], 0.0)
            for pr in range(8):
                xr = xrp.tile([3, 2, 512], F32, tag="xr", name=f"xr{pr}")
                nc.sync.dma_start(
                    xr[:], binf_d[:, pr * 1024:(pr + 1) * 1024]
                    .rearrange("p (s v) -> p s v", s=2))
                ps = psg.tile([128, 1024], F32, tag="big", name=f"rps{pr}")
                for sloc in range(2):
                    nc.tensor.matmul(ps[:, sloc * 512:(sloc + 1) * 512],
                                     wr4[:], xr[:, sloc], start=True, stop=True)
                for sloc in range(2):
                    s = 2 * pr + sloc
                    nc.scalar.activation(
                        box(pad1, s, 1, 1, 1, 1, p0=0, p1=32)[:, 0],
                        ps[0:32, sloc * 512:(sloc + 1) * 512].rearrange(
                            "p (d h w) -> p d h w", d=8, h=8),
                        AF.Copy, accum_out=pt0[0:32, s:s + 1])
                sqt = drainp.tile([128, 1024], BF16, tag="sqt", name=f"sq0_{pr}")
                nc.scalar.activation(sqt[:], ps[:], AF.Square,
                                     accum_out=qt0[:, pr:pr + 1])

            epst = statp.tile([128, 1], F32, tag="epst")
            nc.vector.memset(epst[:], EPS)

            def bn_multi(specs, layer):
                # specs: list of (ptile, qtile, gcol, bcol, stride, off);
                # one AllReduce carries all groups' (sum, sumsq) stats
                ng = len(specs)
                st = statp.tile([128, 2 * ng], F32, tag=f"st{layer}")
                for i, (ptile, qtile, _, _, stride, off) in enumerate(specs):
                    for j, tl in ((0, ptile), (1, qtile)):
                        src = tl[:]
                        if stride != 1:
                            src = tl[:].rearrange(
                                "p (s h) -> p s h", h=stride)[:, :, off]
                        nc.vector.tensor_reduce(
                            st[:, 2 * i + j:2 * i + j + 1], src, axis=AX.X, op=OP.add)
                cc_in = dramp.tile([128, 2 * ng], F32, tag=f"cci{layer}")
                cc_out = dramp.tile([128, 2 * ng], F32, tag=f"cco{layer}")
                nc.sync.dma_start(cc_in[:], st[:])
                if n_cores > 1:
                    nc.gpsimd.collective_compute(
                        "AllReduce", OP.add,
                        replica_groups=[list(range(n_cores))],
                        ins=[cc_in.opt()], outs=[cc_out.opt()])
                else:
                    nc.sync.dma_start(cc_out[:], cc_in[:])
                gst = statp.tile([128, 2 * ng], F32, tag=f"gst{layer}")
                nc.sync.dma_start(gst[:], cc_out[:])
                out = []
                for i, (_, _, gcol, bcol, _, off) in enumerate(specs):
                    mean = statp.tile([128, 1], F32, tag=f"mean{layer}_{i}")
                    var = statp.tile([128, 1], F32, tag=f"var{layer}_{i}")
                    sc = statp.tile([128, 1], F32, tag=f"sc{layer}_{i}")
                    bi = statp.tile([128, 1], F32, tag=f"bi{layer}_{i}")
                    nc.vector.tensor_scalar(mean[:], gst[:, 2 * i:2 * i + 1],
                                            nstat, None, op0=OP.mult)
                    nc.vector.tensor_tensor(var[:], mean[:], mean[:], op=OP.mult)
                    nc.vector.scalar_tensor_tensor(
                        var[:], gst[:, 2 * i + 1:2 * i + 2], nstat, var[:],
                        op0=OP.mult, op1=OP.subtract)
                    nc.scalar.activation(var[:], var[:], AF.Sqrt, bias=epst[:])
                    nc.vector.reciprocal(var[:], var[:])
                    nc.vector.tensor_tensor(sc[:], gb[:, gcol:gcol + 1], var[:], op=OP.mult)
                    nc.vector.tensor_tensor(bi[:], mean[:], sc[:], op=OP.mult)
                    nc.vector.tensor_tensor(bi[:], gb[:, bcol:bcol + 1], bi[:],
                                            op=OP.subtract)
                    out.append((sc, bi))
                return out

            def bn_block(ptile, qtile, gcol, bcol, layer):
                return bn_multi([(ptile, qtile, gcol, bcol, 1, 0)], layer)[0]

            def relu_pass(pad, sc, bi, bsz):
                # y = relu(sc*x + bi) on every center's box, block 0 only;
                # sigma-shifted copies are made afterwards by shift-DMAs
                for s in range(S):
                    v = box(pad, s, 1, 1, 1, 1, p0=0, p1=bsz)[:, 0]
                    nc.scalar.activation(v, v, AF.Relu,
                                         bias=bi[0:bsz], scale=sc[0:bsz])

            sc0, bi0 = bn_block(pt0, qt0, 0, 1, 0)
            relu_pass(pad1, sc0, bi0, 32)
            for t in (1, 2):
                nc.sync.dma_start(pad1[32 * t:32 * (t + 1), 0:PADW - t],
                                  pad1[0:32, t:PADW])

            # ---------------- conv1 (sigma-packed, 9 matmuls/center) -------
            pt1 = statp.tile([128, 16], F32, tag="pt1")
            qt1 = statp.tile([128, 8], F32, tag="qt1")
            nc.vector.memset(pt1[64:128, :], 0.0)
            for prp in range(4):
                pss = [psg.tile([128, 1024], F32, tag="big", name=f"c1ps{2 * prp + i}")
                       for i in range(2)]
                for m, (a, b) in enumerate(C1_MS):
                    lhsT = w1s[:, m * 128:(m + 1) * 128]
                    for i in range(2):
                        for sloc in range(2):
                            rhs = box(pad1, 2 * (2 * prp + i) + sloc, 1, a, b, 0)[:, 0]
                            nc.tensor.matmul(
                                pss[i][:, sloc * 512:(sloc + 1) * 512],
                                lhsT, rhs, start=(m == 0), stop=(m == 8))
                for i in range(2):
                    pr = 2 * prp + i
                    for sloc in range(2):
                        s = 2 * pr + sloc
                        nc.scalar.activation(
                            box(pad2, s, 1, 1, 1, 1, p0=0, p1=64)[:, 0],
                            pss[i][0:64, sloc * 512:(sloc + 1) * 512].rearrange(
                                "p (d h w) -> p d h w", d=8, h=8),
                            AF.Copy, accum_out=pt1[0:64, s:s + 1])
                    sqt = drainp.tile([128, 1024], BF16, tag="sqt", name=f"sq1_{pr}")
                    nc.scalar.activation(sqt[:], pss[i][:], AF.Square,
                                         accum_out=qt1[:, pr:pr + 1])

            sc1, bi1 = bn_block(pt1, qt1, 2, 3, 1)
            relu_pass(pad2, sc1, bi1, 64)
            nc.sync.dma_start(pad2[64:128, 0:PADW - 1], pad2[0:64, 1:PADW])

            # ---------------- conv2 (sigma-packed, 18 matmuls/center) ------
            pt2 = statp.tile([128, 16], F32, tag="pt2")
            qt2 = statp.tile([128, 8], F32, tag="qt2")
            for prp in range(4):
                pss = [psg.tile([128, 1024], F32, tag="big", name=f"c2ps{2 * prp + i}")
                       for i in range(2)]
                for m, (a, b, c0) in enumerate(C2_MS):
                    lhsT = w2s[:, m * 128:(m + 1) * 128]
                    for i in range(2):
                        for sloc in range(2):
                            rhs = box(pad2, 2 * (2 * prp + i) + sloc, 1, a, b, c0)[:, 0]
                            nc.tensor.matmul(
                                pss[i][:, sloc * 512:(sloc + 1) * 512],
                                lhsT, rhs, start=(m == 0), stop=(m == 17))
                for i in range(2):
                    pr = 2 * prp + i
                    for sloc in range(2):
                        s = 2 * pr + sloc
                        nc.scalar.activation(
                            box(pad3, s, 1, 1, 1, 1)[:, 0],
                            pss[i][:, sloc * 512:(sloc + 1) * 512].rearrange(
                                "p (d h w) -> p d h w", d=8, h=8),
                            AF.Copy, accum_out=pt2[:, s:s + 1])
                    sqt = drainp.tile([128, 1024], BF16, tag="sqt", name=f"sq2_{pr}")
                    nc.scalar.activation(sqt[:], pss[i][:], AF.Square,
                                         accum_out=qt2[:, pr:pr + 1])

            sc2, bi2 = bn_block(pt2, qt2, 4, 5, 2)
            relu_pass(pad3, sc2, bi2, 128)

            # ---------------- conv3 (27 taps x 2 halves per center) --------
            pt3 = statp.tile([128, 2 * S], F32, tag="pt3")
            qt3 = statp.tile([128, 2 * S], F32, tag="qt3")
            mxs = statp.tile([128, 2, S], F32, tag="mxs")
            mns = statp.tile([128, 2, S], F32, tag="mns")
            for sp in range(8):
                pss = [psg.tile([128, 1024], F32, tag="big", name=f"c3ps{2 * sp + i}")
                       for i in range(2)]
                for o in range(27):
                    dd, dh, dw = o // 9, (o // 3) % 3, o % 3
                    for h in range(2):
                        lhsT = w3p[:, (o * 2 + h) * 128:(o * 2 + h + 1) * 128]
                        for i in range(2):
                            rhs = box(pad3, 2 * sp + i, 1, dd, dh, dw)[:, 0]
                            nc.tensor.matmul(pss[i][:, h * 512:(h + 1) * 512],
                                             lhsT, rhs, start=(o == 0), stop=(o == 26))
                for i in range(2):
                    s = 2 * sp + i
                    nc.vector.tensor_reduce(
                        mxs[:, :, s:s + 1],
                        pss[i][:].rearrange("p (h v) -> p h v", h=2),
                        axis=AX.X, op=OP.max)
                    nc.vector.tensor_reduce(
                        mns[:, :, s:s + 1],
                        pss[i][:].rearrange("p (h v) -> p h v", h=2),
                        axis=AX.X, op=OP.min)
                    for h in range(2):
                        sqa = drainp.tile([128, 512], BF16, tag="sqf", name=f"sqf_{s}_{h}")
                        nc.scalar.activation(sqa[:], pss[i][:, h * 512:(h + 1) * 512],
                                             AF.Copy,
                                             accum_out=pt3[:, s * 2 + h:s * 2 + h + 1])
                        sqb = drainp.tile([128, 512], BF16, tag="s2q", name=f"s2q_{s}_{h}")
                        nc.scalar.activation(sqb[:], pss[i][:, h * 512:(h + 1) * 512],
                                             AF.Square,
                                             accum_out=qt3[:, s * 2 + h:s * 2 + h + 1])

            desc_sb = statp.tile([128, 2, S], F32, tag="descsb")
            bn3 = bn_multi([(pt3, qt3, 6, 7, 2, 0), (pt3, qt3, 8, 9, 2, 1)], 3)
            for h in range(2):
                sc3, bi3 = bn3[h]
                zmx = statp.tile([128, S], F32, tag=f"zmx{h}")
                zmn = statp.tile([128, S], F32, tag=f"zmn{h}")
                nc.vector.tensor_scalar(zmx[:], mxs[:, h, :], sc3[:], bi3[:],
                                        op0=OP.mult, op1=OP.add)
                nc.vector.tensor_scalar(zmn[:], mns[:, h, :], sc3[:], bi3[:],
                                        op0=OP.mult, op1=OP.add)
                csel = statp.tile([128, 1], F32, tag=f"csel{h}")
                nc.vector.tensor_scalar(csel[:], sc3[:], 0.0, None, op0=OP.is_ge)
                nc.vector.tensor_tensor(zmx[:], zmx[:], zmn[:], op=OP.subtract)
                nc.vector.scalar_tensor_tensor(zmx[:], zmx[:], csel[:], zmn[:],
                                               op0=OP.mult, op1=OP.add)
                nc.vector.tensor_scalar(desc_sb[:, h, :], zmx[:], 0.0, None, op0=OP.max)

            for h in range(2):
                nc.sync.dma_start(
                    desc_d.ap().rearrange("s (h ch) -> ch h s", h=2)[:, h, :],
                    desc_sb[:, h, :])

    nc.compile()
    return nc


def _host_pack(inputs):
    pts_all = np.asarray(inputs["points"], np.float32)
    ctr_all = np.asarray(inputs["center_points"], np.float32)
    w_raise = np.asarray(inputs["w_raise"], np.float32)
    w1 = np.asarray(inputs["w1"], np.float32)
    w2 = np.asarray(inputs["w2"], np.float32)
    w3 = np.asarray(inputs["w3"], np.float32)

    wr4 = np.zeros((3, 128), np.float32)
    for g in range(4):
        wr4[:, g * 32:(g + 1) * 32] = w_raise.T
    gb = np.zeros((128, 10), np.float32)
    g0 = np.asarray(inputs["g0"], np.float32); be0 = np.asarray(inputs["be0"], np.float32)
    g1 = np.asarray(inputs["g1"], np.float32); be1 = np.asarray(inputs["be1"], np.float32)
    g2 = np.asarray(inputs["g2"], np.float32); be2 = np.asarray(inputs["be2"], np.float32)
    g3 = np.asarray(inputs["g3"], np.float32); be3 = np.asarray(inputs["be3"], np.float32)
    gb[:, 0] = np.tile(g0, 4); gb[:, 1] = np.tile(be0, 4)
    gb[:, 2] = np.tile(g1, 2); gb[:, 3] = np.tile(be1, 2)
    gb[:, 4] = g2; gb[:, 5] = be2
    gb[:, 6] = g3[:128]; gb[:, 7] = be3[:128]
    gb[:, 8] = g3[128:]; gb[:, 9] = be3[128:]

    aux = np.zeros(AUX_F32, np.float32)
    aux[0:384] = wr4.reshape(-1)
    aux[384:1664] = gb.reshape(-1)
    aux[1664:1673] = [-1e30] + R2_B + [3e38]
    aux[1673:1682] = [4.0] + COS_B + [-4.0]
    aux[1682:1690] = [0, 0, 0, 0, 1, 1, 1, 1]
    aux[1690:1698] = [0, 0, 1, 1, 1, 1, 0, 0]
    aux[1698:1706] = [0, 1, 1, 0, 0, 1, 1, 0]

    blob = np.empty(BLOB_EL, ml_dtypes.bfloat16)
    blob[0:AUX_EL] = aux.view(ml_dtypes.bfloat16)
    blob[OFF_W1:OFF_W2] = np.ascontiguousarray(
        w1.reshape(64, 32, 27).transpose(1, 2, 0)).astype(ml_dtypes.bfloat16).reshape(-1)
    blob[OFF_W2:OFF_W3] = np.ascontiguousarray(
        w2.reshape(128, 64, 27).transpose(1, 2, 0)).astype(ml_dtypes.bfloat16).reshape(-1)
    blob[OFF_W3:BLOB_EL] = np.ascontiguousarray(
        w3.reshape(2, 128, 128, 27).transpose(2, 3, 0, 1)).astype(
            ml_dtypes.bfloat16).reshape(-1)

    in_maps = []
    for c in range(NCORES):
        b = c // 4
        pts = np.ascontiguousarray(np.transpose(pts_all[b].reshape(NCH, 128, 3), (1, 0, 2)))
        s0 = (c % 4) * S
        ctrv = np.ascontiguousarray(ctr_all[b, s0:s0 + S, :].reshape(-1))
        shard = np.ascontiguousarray(blob[c * SHARD_EL:(c + 1) * SHARD_EL])
        in_maps.append(dict(pts=pts, ctrv=ctrv, blob=shard))
    return in_maps


_CACHED_NC = None


def _get_nc():
    global _CACHED_NC
    if _CACHED_NC is None:
        _CACHED_NC = _build_nc(NCORES)
    return _CACHED_NC


def kernel(**inputs) -> np.ndarray:
    nc = _get_nc()
    in_maps = _host_pack(inputs)
    res = bass_utils.run_bass_kernel_spmd(nc, in_maps, core_ids=list(range(NCORES)))
    out = np.concatenate([np.asarray(res.results[c]["desc"], np.float32)
                          for c in range(NCORES)], axis=0)     # (128, 256)
    return out.reshape(B, M, 256)
